# revision 10
# baseline (speedup 1.0000x reference)
"""Trainium2 Bass kernel for nn_CrossDimensionalGraphNet (hetero-GAT on a fixed
Catan-style board graph), batch-data-parallel over 8 NeuronCores.

Design:
  - batch on SBUF partitions (128/half, 2 halves per core, 8 cores x 256 = 2048)
  - feature matmuls as x-as-lhsT chunks: per (node, half) LDW a [65,128]
    transposed-input tile (64 feats + ones row) and one matmul against a
    packed weight rhs producing, in batch layout: hs per relation, the lin
    skip output (bias via ones-row), and the attention vectors a_src/a_dst.
  - attention softmax (exact GATConv, incl. leaky-relu) computed per edge-slot
    in a (dst, k)-padded layout; the irregular gathers are compiled into
    piecewise-affine access-pattern runs (beam-search-minimized).
  - messages alpha*hs via bf16 tensor_tensor in 2x mode (alpha pre-duplicated
    into bf16 pairs so both operands have unit-stride innermost dims).
  - unnormalized-exp trick: no segment_max (mathematically identical softmax);
    1/denom folded into alpha; relation mean-weights and all biases folded
    into the packed weights host-side.
"""

import numpy as np
from itertools import permutations, combinations

import concourse.bass as bass
import concourse.mybir as mybir
from concourse.tile import TileContext
from concourse.bass_utils import run_bass_kernel_spmd

try:
    import ml_dtypes
    BF16 = ml_dtypes.bfloat16
except ImportError:  # pragma: no cover
    import jax.numpy as jnp
    BF16 = jnp.bfloat16

# ----------------------------------------------------------------------------
# Fixed graph (hardcoded; identical to the reference problem definition)
# ----------------------------------------------------------------------------
HEX_N, VER_N, ROAD_N = 19, 54, 72
D = 64
B_CORE = 256            # batch per core
P = 128                 # partitions (batch-half)
N_CORES = 8
SLOPE = 0.2

_H2V0 = [0, 2, 3, 0, 1, 2, 0, 1, 6, 0, 5, 6, 0, 4, 5, 0, 3, 4, 1, 2, 8, 1, 7, 8, 1, 7, 18, 1, 6, 18, 2, 9, 10, 2, 8, 9, 2, 3, 10, 3, 10, 11, 3, 4, 12, 3, 11, 12, 4, 5, 14, 4, 13, 14, 4, 12, 13, 5, 6, 16, 5, 15, 16, 5, 14, 15, 6, 17, 18, 6, 16, 17, 7, 8, 7, 7, 7, 18, 8, 9, 8, 9, 9, 9, 10, 10, 10, 11, 11, 11, 12, 11, 12, 13, 12, 13, 14, 13, 13, 14, 15, 14, 15, 16, 15, 15, 16, 17, 16, 17, 18, 17, 17, 18]
_H2V1 = [0, 0, 0, 1, 1, 1, 2, 2, 2, 3, 3, 3, 4, 4, 4, 5, 5, 5, 6, 6, 6, 7, 7, 7, 8, 8, 8, 9, 9, 9, 10, 10, 10, 11, 11, 11, 12, 12, 12, 13, 13, 13, 14, 14, 14, 15, 15, 15, 16, 16, 16, 17, 17, 17, 18, 18, 18, 19, 19, 19, 20, 20, 20, 21, 21, 21, 22, 22, 22, 23, 23, 23, 24, 24, 25, 26, 27, 27, 28, 28, 29, 30, 31, 32, 32, 33, 34, 34, 35, 36, 36, 37, 38, 38, 39, 40, 40, 41, 42, 43, 43, 44, 45, 45, 46, 47, 48, 48, 49, 50, 50, 51, 52, 53]
_R2R0 = [0, 1, 0, 2, 1, 2, 0, 3, 0, 4, 3, 4, 3, 5, 3, 6, 5, 6, 5, 7, 5, 8, 7, 8, 7, 9, 7, 10, 9, 10, 1, 9, 1, 11, 9, 11, 4, 12, 4, 13, 12, 13, 12, 14, 12, 15, 14, 15, 14, 16, 14, 17, 16, 17, 6, 16, 6, 18, 16, 18, 19, 20, 19, 21, 20, 21, 13, 19, 13, 22, 19, 22, 2, 20, 2, 23, 20, 23, 23, 24, 23, 25, 24, 25, 11, 26, 11, 27, 26, 27, 24, 26, 24, 28, 26, 28, 10, 29, 10, 30, 29, 30, 29, 31, 29, 32, 31, 32, 27, 31, 27, 33, 31, 33, 8, 34, 8, 35, 34, 35, 34, 36, 34, 37, 36, 37, 30, 36, 30, 38, 36, 38, 18, 39, 18, 40, 39, 40, 35, 39, 35, 41, 39, 41, 15, 42, 15, 43, 42, 43, 42, 44, 44, 45, 17, 45, 17, 46, 45, 46, 22, 47, 22, 48, 47, 48, 43, 47, 49, 50, 48, 49, 21, 50, 21, 51, 50, 51, 51, 52, 25, 52, 25, 53, 52, 53, 53, 54, 28, 55, 28, 56, 55, 56, 54, 55, 33, 57, 33, 58, 57, 58, 56, 57, 32, 59, 32, 60, 59, 60, 59, 61, 58, 61, 38, 62, 38, 63, 62, 63, 60, 62, 37, 64, 37, 65, 64, 65, 64, 66, 63, 66, 41, 67, 41, 68, 67, 68, 65, 67, 40, 69, 40, 70, 69, 70, 69, 71, 68, 71, 46, 70]
_R2R1 = [1, 0, 2, 0, 2, 1, 3, 0, 4, 0, 4, 3, 5, 3, 6, 3, 6, 5, 7, 5, 8, 5, 8, 7, 9, 7, 10, 7, 10, 9, 9, 1, 11, 1, 11, 9, 12, 4, 13, 4, 13, 12, 14, 12, 15, 12, 15, 14, 16, 14, 17, 14, 17, 16, 16, 6, 18, 6, 18, 16, 20, 19, 21, 19, 21, 20, 19, 13, 22, 13, 22, 19, 20, 2, 23, 2, 23, 20, 24, 23, 25, 23, 25, 24, 26, 11, 27, 11, 27, 26, 26, 24, 28, 24, 28, 26, 29, 10, 30, 10, 30, 29, 31, 29, 32, 29, 32, 31, 31, 27, 33, 27, 33, 31, 34, 8, 35, 8, 35, 34, 36, 34, 37, 34, 37, 36, 36, 30, 38, 30, 38, 36, 39, 18, 40, 18, 40, 39, 39, 35, 41, 35, 41, 39, 42, 15, 43, 15, 43, 42, 44, 42, 45, 44, 45, 17, 46, 17, 46, 45, 47, 22, 48, 22, 48, 47, 47, 43, 50, 49, 49, 48, 50, 21, 51, 21, 51, 50, 52, 51, 52, 25, 53, 25, 53, 52, 54, 53, 55, 28, 56, 28, 56, 55, 55, 54, 57, 33, 58, 33, 58, 57, 57, 56, 59, 32, 60, 32, 60, 59, 61, 59, 61, 58, 62, 38, 63, 38, 63, 62, 62, 60, 64, 37, 65, 37, 65, 64, 66, 64, 66, 63, 67, 41, 68, 41, 68, 67, 67, 65, 69, 40, 70, 40, 70, 69, 71, 69, 71, 68, 70, 46]
_R2V0 = [0, 1, 2, 0, 3, 4, 3, 5, 6, 5, 7, 8, 7, 9, 10, 1, 9, 11, 4, 12, 13, 12, 14, 15, 14, 16, 17, 6, 16, 18, 19, 20, 21, 13, 19, 22, 2, 20, 23, 23, 24, 25, 11, 26, 27, 24, 26, 28, 10, 29, 30, 29, 31, 32, 27, 31, 33, 8, 34, 35, 34, 36, 37, 30, 36, 38, 18, 39, 40, 35, 39, 41, 15, 42, 43, 42, 44, 44, 45, 17, 45, 46, 22, 47, 48, 43, 47, 49, 50, 48, 49, 21, 50, 51, 51, 52, 25, 52, 53, 53, 54, 28, 55, 56, 54, 55, 33, 57, 58, 56, 57, 32, 59, 60, 59, 61, 58, 61, 38, 62, 63, 60, 62, 37, 64, 65, 64, 66, 63, 66, 41, 67, 68, 65, 67, 40, 69, 70, 69, 71, 68, 71, 46, 70]
_R2V1 = [0, 0, 0, 1, 1, 1, 2, 2, 2, 3, 3, 3, 4, 4, 4, 5, 5, 5, 6, 6, 6, 7, 7, 7, 8, 8, 8, 9, 9, 9, 10, 10, 10, 11, 11, 11, 12, 12, 12, 13, 13, 13, 14, 14, 14, 15, 15, 15, 16, 16, 16, 17, 17, 17, 18, 18, 18, 19, 19, 19, 20, 20, 20, 21, 21, 21, 22, 22, 22, 23, 23, 23, 24, 24, 24, 25, 25, 26, 26, 27, 27, 27, 28, 28, 28, 29, 29, 30, 30, 31, 31, 32, 32, 32, 33, 33, 34, 34, 34, 35, 35, 36, 36, 36, 37, 37, 38, 38, 38, 39, 39, 40, 40, 40, 41, 41, 42, 42, 43, 43, 43, 44, 44, 45, 45, 45, 46, 46, 47, 47, 48, 48, 48, 49, 49, 50, 50, 50, 51, 51, 52, 52, 53, 53]
_V2V0 = [0, 0, 0, 1, 1, 1, 2, 2, 2, 3, 3, 3, 4, 4, 4, 5, 5, 5, 6, 6, 6, 7, 7, 7, 8, 8, 8, 9, 9, 9, 10, 10, 10, 11, 11, 11, 12, 12, 12, 13, 13, 13, 14, 14, 14, 15, 15, 15, 16, 16, 16, 17, 17, 17, 18, 18, 18, 19, 19, 19, 20, 20, 20, 21, 21, 21, 22, 22, 22, 23, 23, 23, 24, 24, 24, 25, 25, 26, 26, 27, 27, 27, 28, 28, 28, 29, 29, 30, 30, 31, 31, 32, 32, 32, 33, 33, 34, 34, 34, 35, 35, 36, 36, 36, 37, 37, 38, 38, 38, 39, 39, 40, 40, 40, 41, 41, 42, 42, 43, 43, 43, 44, 44, 45, 45, 45, 46, 46, 47, 47, 48, 48, 48, 49, 49, 50, 50, 50, 51, 51, 52, 52, 53, 53]
_V2V1 = [1, 5, 12, 0, 2, 6, 1, 3, 9, 2, 4, 19, 3, 5, 16, 0, 4, 14, 1, 7, 11, 6, 8, 24, 7, 9, 27, 2, 8, 22, 11, 12, 32, 6, 10, 28, 0, 10, 13, 12, 15, 34, 5, 15, 18, 13, 14, 36, 4, 17, 21, 16, 18, 40, 14, 17, 38, 3, 20, 23, 19, 21, 45, 16, 20, 43, 9, 23, 50, 19, 22, 48, 7, 25, 29, 24, 26, 25, 27, 8, 26, 53, 11, 29, 31, 24, 28, 31, 32, 28, 30, 10, 30, 33, 32, 34, 13, 33, 35, 34, 37, 15, 37, 39, 35, 36, 18, 39, 42, 36, 38, 17, 41, 44, 40, 42, 38, 41, 21, 44, 47, 40, 43, 20, 46, 49, 45, 47, 43, 46, 23, 49, 52, 45, 48, 22, 51, 53, 50, 52, 48, 51, 27, 50]

_EDGES = {
    'h2v': (np.array(_H2V0), np.array(_H2V1)),      # src hex -> dst vertex
    'v2h': (np.array(_H2V1), np.array(_H2V0)),      # src vertex -> dst hex
    'r2v': (np.array(_R2V0), np.array(_R2V1)),
    'v2r': (np.array(_R2V1), np.array(_R2V0)),
    'v2v': (np.array(_V2V0), np.array(_V2V1)),
    'r2r': (np.array(_R2R0), np.array(_R2R1)),
}

_NN = {'h': HEX_N, 'v': VER_N, 'r': ROAD_N}

# relation meta: (src_type, dst_type, hs_offset_in_pack, ws_vec_idx, wd_vec_idx)
# pack layouts (columns of the rhs weight pack, per src type):
#  h: hs_h2v[0:64]  lin_h[64:128]  vecs[128:130] = [ws_h2v, wd_v2h]      N=130
#  v: hs_v2h[0:64]  hs_v2r[64:128] hs_v2v[128:192] lin_v[192:256]
#     vecs[256:262] = [ws_v2h, ws_v2r, ws_v2v, wd_h2v, wd_r2v, wd_v2v]   N=262
#  r: hs_r2v[0:64]  hs_r2r[64:128] lin_r[128:192]
#     vecs[192:196] = [ws_r2v, ws_r2r, wd_v2r, wd_r2r]                   N=196
_PACK = {
    'h': dict(N=130, hs_w=64,  lin0=64,  vec0=128, nvec=2),
    'v': dict(N=262, hs_w=192, lin0=192, vec0=256, nvec=6),
    'r': dict(N=196, hs_w=128, lin0=128, vec0=192, nvec=4),
}
_RELS = {
    'h2v': dict(src='h', dst='v', hoff=0,   ws=0, wd=3, scale=1.0 / 3),
    'r2v': dict(src='r', dst='v', hoff=0,   ws=0, wd=4, scale=1.0 / 3),
    'v2v': dict(src='v', dst='v', hoff=128, ws=2, wd=5, scale=1.0 / 3),
    'v2h': dict(src='v', dst='h', hoff=0,   ws=0, wd=1, scale=1.0),
    'v2r': dict(src='v', dst='r', hoff=64,  ws=1, wd=2, scale=0.5),
    'r2r': dict(src='r', dst='r', hoff=64,  ws=1, wd=3, scale=0.5),
}
_REL_ORDER = ['h2v', 'r2v', 'v2v', 'v2h', 'v2r', 'r2r']


# ----------------------------------------------------------------------------
# Slot-table construction: per relation choose idx[d,k] assignment minimizing
# the number of affine runs per k column (beam search), then extract runs.
# ----------------------------------------------------------------------------

def _beam_assign(nbrs, Ns, K, width=300):
    beam = [(0, tuple([None] * K), [])]
    for d in range(len(nbrs)):
        ns = list(nbrs[d])
        deg = len(ns)
        opts = [(sl, pm) for sl in combinations(range(K), deg)
                for pm in permutations(ns)]
        cand = {}
        for (cost, states, assigns) in beam:
            for slots, perm in opts:
                assign = dict(zip(slots, perm))
                ncost, nstates, row = cost, [], []
                for k in range(K):
                    st = states[k]
                    if k in assign:
                        v = assign[k]
                        if st is None:
                            ncost += 1
                            nstates.append((v, None))
                        else:
                            prev, stride = st
                            if stride is None:
                                nstates.append((v, v - prev))
                            elif v == prev + stride:
                                nstates.append((v, stride))
                            else:
                                ncost += 1
                                nstates.append((v, None))
                        row.append(v)
                    else:
                        if st is None:
                            nstates.append(None)
                        else:
                            prev, stride = st
                            if stride is None:
                                nstates.append((prev, stride))
                            else:
                                v = prev + stride
                                nstates.append((v, stride) if 0 <= v < Ns else None)
                        row.append(-1)
                key = (ncost, tuple(nstates))
                if key not in cand:
                    cand[key] = (ncost, tuple(nstates), assigns + [row])
        beam = sorted(cand.values(), key=lambda x: x[0])[:width]
    return np.array(beam[0][2])


def _extract_runs(col, Ns):
    """-> list of (d0, length, v0, stride); wildcards (-1) filled affinely."""
    Nd = len(col)
    runs, d = [], 0
    while d < Nd:
        j, pts, stride = d, [], None
        while j < Nd:
            v = col[j]
            if v >= 0:
                if not pts:
                    pts.append((j, v))
                else:
                    p0, v0 = pts[0]
                    if stride is None:
                        num, den = v - v0, j - p0
                        if num % den == 0 and all(
                                0 <= v0 + (num // den) * (p - p0) < Ns
                                for p in range(d, j + 1)):
                            stride = num // den
                            pts.append((j, v))
                        else:
                            break
                    elif v == pts[0][1] + stride * (j - pts[0][0]):
                        pts.append((j, v))
                    else:
                        break
            else:
                if pts and stride is not None:
                    if not (0 <= pts[0][1] + stride * (j - pts[0][0]) < Ns):
                        break
            j += 1
        ln = j - d
        if not pts:
            runs.append((d, ln, 0, 0))
        else:
            p0, v0 = pts[0]
            st = stride if stride is not None else 0
            start = v0 - st * (p0 - d)
            if 0 <= start < Ns and 0 <= start + st * (ln - 1) < Ns:
                runs.append((d, ln, start, st))
            else:
                if p0 > d:
                    runs.append((d, p0 - d, v0, 0))
                runs.append((p0, j - p0, v0, st))
        d = j
    return runs


def _pad_runs(idx_col):
    """maximal stretches of pad (-1) positions -> list of (d0, length)."""
    out, d = [], 0
    Nd = len(idx_col)
    while d < Nd:
        if idx_col[d] < 0:
            j = d
            while j < Nd and idx_col[j] < 0:
                j += 1
            out.append((d, j - d))
            d = j
        else:
            d += 1
    return out


def _build_meta():
    meta = {}
    for name, (src, dst) in _EDGES.items():
        Ns = _NN[_RELS[name]['src']]
        Nd = _NN[_RELS[name]['dst']]
        nbrs = [[] for _ in range(Nd)]
        for s, d in zip(src, dst):
            nbrs[d].append(int(s))
        K = max(len(x) for x in nbrs)
        idx = _beam_assign(nbrs, Ns, K)
        runs = [_extract_runs(idx[:, k], Ns) for k in range(K)]
        pads = [_pad_runs(idx[:, k]) for k in range(K)]
        meta[name] = dict(K=K, Nd=Nd, Ns=Ns, idx=idx, runs=runs, pads=pads)
    return meta


_META = None


def _meta():
    global _META
    if _META is None:
        _META = _build_meta()
    return _META


# ----------------------------------------------------------------------------
# Bass program
# ----------------------------------------------------------------------------

def _ap_run(tile_ap, off, step, n, inner=None):
    """Build an AP over a [P, W] tile: partition dim + [step,n] + inner dims.
    off/step in elements of the tile dtype. inner: list of [step, count]."""
    ap = [list(tile_ap.ap[0]), [int(step), int(n)]]
    if inner:
        ap += [list(x) for x in inner]
    return bass.AP(tile_ap.tensor, tile_ap.offset + int(off), ap)


def _split_multi_waits(nc):
    """This toolchain's walrus accepts only ONE sync-wait per queue
    instruction (MM/AC/NO structs). Tile emits multi-waits freely, so move
    extras onto same-engine NoOps inserted immediately before — semantically
    identical (engine queues execute in order)."""
    eng_map = {
        mybir.EngineType.PE: nc.tensor,
        mybir.EngineType.Activation: nc.scalar,
        mybir.EngineType.DVE: nc.vector,
        mybir.EngineType.Pool: nc.gpsimd,
        mybir.EngineType.SP: nc.sync,
    }
    # plan first: (block, ordered list of (inst_name, extra_waits))
    plans = []
    for fn in nc.m.functions:
        for bb in fn.blocks:
            plan = []
            for inst in bb.instructions:
                si = inst.sync_info
                waits = list(si.on_wait) if si is not None and si.on_wait else []
                if (len(waits) > 1 and inst.opcode != 'EventSemaphore'
                        and inst.engine in eng_map):
                    plan.append((inst.name, inst.engine, waits))
            if plan:
                plans.append((bb, plan))
    # create all nops (they auto-append to the current block; strip after)
    created = {}
    nop_names = set()
    for bb, plan in plans:
        for name, engine, waits in plan:
            nops = []
            for w in waits[:-1]:
                nop = eng_map[engine].nop(hint='waitsplit').ins
                nop.sync_info = mybir.SyncInfo(on_wait=[w], on_update=[])
                nops.append(nop)
                nop_names.add(nop.name)
            created[name] = nops
    # strip auto-appended copies from every block, then splice
    total = 0
    for fn in nc.m.functions:
        for bb in fn.blocks:
            insts = [i for i in bb.instructions if i.name not in nop_names]
            out = []
            for inst in insts:
                if inst.name in created:
                    out.extend(created[inst.name])
                    total += len(created[inst.name])
                    si = inst.sync_info
                    si.on_wait = [list(si.on_wait)[-1]]
                    inst.sync_info = si
                out.append(inst)
            l = bb.instructions
            l.clear()
            l.extend(out)
    return total


def build_nc():
    meta = _meta()
    FP32 = mybir.dt.float32
    BF = mybir.dt.bfloat16
    nc = bass.Bass()

    xt = {t: nc.declare_dram_parameter(f'xt_{t}', [65, _NN[t] * B_CORE], BF, isOutput=False)
          for t in ('h', 'v', 'r')}
    wp = {t: nc.declare_dram_parameter(f'wp_{t}', [65, _PACK[t]['N']], BF, isOutput=False)
          for t in ('h', 'v', 'r')}
    out_d = {t: nc.declare_dram_parameter(f'out_{t}', [B_CORE, _NN[t] * D], FP32, isOutput=True)
             for t in ('h', 'v', 'r')}

    with TileContext(nc) as tc:
        with (
            tc.tile_pool(name='const', bufs=1) as cpool,
            tc.tile_pool(name='psum', bufs=2, space='PSUM') as ppool,
            tc.tile_pool(name='sb', bufs=1) as sbpool,
            tc.tile_pool(name='small', bufs=2) as smpool,
            tc.tile_pool(name='msg', bufs=4) as mpool,
            tc.tile_pool(name='outp', bufs=1) as opool,
        ):
            # constants
            xt_sb, wp_sb = {}, {}
            for t in ('h', 'v', 'r'):
                xt_sb[t] = cpool.tile([65, _NN[t] * B_CORE], BF, tag=f'xt_{t}', name=f'xt_sb_{t}')
                nc.sync.dma_start(out=xt_sb[t][:], in_=xt[t][:])
                wp_sb[t] = cpool.tile([65, _PACK[t]['N']], BF, tag=f'wp_{t}', name=f'wp_sb_{t}')
                nc.sync.dma_start(out=wp_sb[t][:], in_=wp[t][:])

            for half in range(2):
                # ---------------- phase 1: chunk matmuls + staging copies ----
                SB, A, OUT = {}, {}, {}
                for t in ('h', 'v', 'r'):
                    pk = _PACK[t]
                    SB[t] = sbpool.tile([P, _NN[t] * pk['hs_w']], BF, tag=f'SB_{t}', name=f'SB_{t}')
                    A[t] = sbpool.tile([P, _NN[t] * pk['nvec']], FP32, tag=f'A_{t}', name=f'A_{t}')
                    OUT[t] = opool.tile([P, _NN[t] * D], BF, tag=f'OUT_{t}', name=f'OUT_{t}')

                cpy_i = 0
                for t in ('h', 'v', 'r'):
                    pk = _PACK[t]
                    N, hs_w, lin0, vec0, nvec = (pk['N'], pk['hs_w'], pk['lin0'],
                                                 pk['vec0'], pk['nvec'])
                    Ns = _NN[t]
                    for s0 in range(0, Ns, 4):
                        nb = min(4, Ns - s0)
                        mega = ppool.tile([P, 2048], FP32, tag='mega', name='mega')
                        for i in range(nb):
                            s = s0 + i
                            col0 = s * B_CORE + half * P
                            nc.tensor.matmul(
                                out=mega[:, i * 512: i * 512 + N],
                                lhsT=xt_sb[t][:, col0: col0 + P],
                                rhs=wp_sb[t][:],
                                start=True, stop=True)
                        # staged copies: hs -> SB (bf16), lin -> OUT (bf16),
                        # vecs -> A (f32)
                        # stable engine assignment keeps WAW deps
                        # same-engine (avoids multi-sem wait blowup):
                        # hs -> DVE, lin+vecs -> ACT
                        mt = mega[:]
                        if hs_w:
                            src = _ap_run(mt, 0, 512, nb, [[1, hs_w]])
                            dst = _ap_run(SB[t][:], s0 * hs_w, hs_w, nb, [[1, hs_w]])
                            nc.vector.tensor_copy(out=dst, in_=src)
                        src = _ap_run(mt, lin0, 512, nb, [[1, D]])
                        dst = _ap_run(OUT[t][:], s0 * D, D, nb, [[1, D]])
                        nc.vector.tensor_copy(out=dst, in_=src)
                        src = _ap_run(mt, vec0, 512, nb, [[1, nvec]])
                        dst = _ap_run(A[t][:], s0 * nvec, nvec, nb, [[1, nvec]])
                        nc.vector.tensor_copy(out=dst, in_=src)

                # ---------------- phases 2+3 per relation --------------------
                for rel in _REL_ORDER:
                    rm = meta[rel]
                    ri = _RELS[rel]
                    K, Nd, Ns = rm['K'], rm['Nd'], rm['Ns']
                    slots = Nd * K
                    st_, dt_ = ri['src'], ri['dst']
                    nvec_s, nvec_d = _PACK[st_]['nvec'], _PACK[dt_]['nvec']
                    hs_w = _PACK[st_]['hs_w']

                    e = smpool.tile([P, slots], FP32, tag='e')
                    tmp = smpool.tile([P, slots], FP32, tag='tmp')
                    dn = smpool.tile([P, Nd], FP32, tag='dn')
                    rp = smpool.tile([P, Nd], FP32, tag='rp')
                    a2 = smpool.tile([P, slots * 2], BF, tag='a2')

                    ev = e[:].rearrange('p (d k) -> p d k', k=K)
                    # e[d,k] = a_src[idx[d,k]] + a_dst[d]  (per-run TT adds)
                    for k in range(K):
                        for (d0, ln, v0, stp) in rm['runs'][k]:
                            in0 = _ap_run(A[st_][:], v0 * nvec_s + ri['ws'],
                                          stp * nvec_s, ln)
                            in1 = _ap_run(A[dt_][:], d0 * nvec_d + ri['wd'],
                                          nvec_d, ln)
                            out = _ap_run(e[:], d0 * K + k, K, ln)
                            nc.vector.tensor_tensor(out=out, in0=in0, in1=in1,
                                                    op=mybir.AluOpType.add)
                    # leaky relu: e = max(e, 0.2e); then exp on ACT
                    nc.vector.tensor_scalar_mul(tmp[:, :slots], e[:], SLOPE)
                    nc.vector.tensor_tensor(out=e[:], in0=e[:], in1=tmp[:, :slots],
                                            op=mybir.AluOpType.max)
                    nc.scalar.activation(out=e[:], in_=e[:],
                                         func=mybir.ActivationFunctionType.Exp)
                    # zero padded slots
                    for k in range(K):
                        for (d0, ln) in rm['pads'][k]:
                            nc.vector.memset(_ap_run(e[:], d0 * K + k, K, ln), 0.0)
                    # denom + reciprocal; alpha = ex * r[d] (fold 1/denom)
                    nc.vector.tensor_reduce(out=dn[:, :, None], in_=ev,
                                            axis=mybir.AxisListType.X,
                                            op=mybir.AluOpType.add)
                    nc.vector.reciprocal(rp[:], dn[:])
                    rpb = rp[:].rearrange('p (d one) -> p d one', one=1) \
                               .to_broadcast([P, Nd, K])
                    nc.vector.tensor_tensor(out=ev, in0=ev, in1=rpb,
                                            op=mybir.AluOpType.mult)
                    # alpha -> bf16 pairs
                    a2v = a2[:].rearrange('p (s two) -> p s two', two=2)
                    ein = e[:].rearrange('p (s one) -> p s one', one=1) \
                              .to_broadcast([P, slots, 2])
                    nc.vector.tensor_copy(out=a2v, in_=ein)

                    # messages per k: msg_k[d,:] = hs[idx[d,k],:] * alpha[d,k];
                    # interleave k-sum adds so at most ~3 msg tiles are live
                    acc = None
                    for k in range(K):
                        mk = mpool.tile([P, Nd * D], BF, tag='msg', name='msg')
                        for (d0, ln, v0, stp) in rm['runs'][k]:
                            in0 = _ap_run(SB[st_][:], v0 * hs_w + ri['hoff'],
                                          stp * hs_w, ln, [[2, 32], [1, 2]])
                            in1 = _ap_run(a2[:], (d0 * K + k) * 2, K * 2, ln,
                                          [[0, 32], [1, 2]])
                            out = _ap_run(mk[:], d0 * D, D, ln, [[2, 32], [1, 2]])
                            nc.vector.tensor_tensor(out=out, in0=in0, in1=in1,
                                                    op=mybir.AluOpType.mult)
                        if acc is None:
                            acc = mk
                        else:
                            nc.vector.tensor_tensor(out=acc[:], in0=acc[:],
                                                    in1=mk[:],
                                                    op=mybir.AluOpType.add)
                    nc.vector.tensor_tensor(out=OUT[dt_][:], in0=OUT[dt_][:],
                                            in1=acc[:], op=mybir.AluOpType.add)

                # ---------------- phase 4: output DMA (bf16 -> f32 cast) -----
                for t in ('h', 'v', 'r'):
                    nc.gpsimd.dma_start(
                        out=out_d[t][half * P:(half + 1) * P, :],
                        in_=OUT[t][:])
    _split_multi_waits(nc)
    return nc


_NC = None


def _get_nc():
    global _NC
    if _NC is None:
        _NC = build_nc()
    return _NC


# ----------------------------------------------------------------------------
# Host side
# ----------------------------------------------------------------------------

def _np(x):
    return np.asarray(x, dtype=np.float32)


def _build_packs(params):
    """-> dict of bf16 [65, N] weight packs with all folds applied."""
    wsv = {r: _np(params[r]['Ws']) @ _np(params[r]['as']) for r in _RELS}
    wdv = {r: _np(params[r]['Wd']) @ _np(params[r]['ad']) for r in _RELS}
    Ws = {r: _np(params[r]['Ws']) for r in _RELS}
    b = {r: _np(params[r]['b']) for r in _RELS}
    lin = {t: (_np(params[f'lin_{t}']['W']), _np(params[f'lin_{t}']['b']))
           for t in ('h', 'v', 'r')}
    bias = {
        'h': b['v2h'] + lin['h'][1],
        'v': (b['h2v'] + b['r2v'] + b['v2v']) / 3.0 + lin['v'][1],
        'r': (b['v2r'] + b['r2r']) / 2.0 + lin['r'][1],
    }
    packs = {}
    specs = {
        'h': (['h2v'], ['h2v', 'v2h']),
        'v': (['v2h', 'v2r', 'v2v'], ['v2h', 'v2r', 'v2v', 'h2v', 'r2v', 'v2v']),
        'r': (['r2v', 'r2r'], ['r2v', 'r2r', 'v2r', 'r2r']),
    }
    for t, (hs_rels, vec_rels) in specs.items():
        pk = _PACK[t]
        w = np.zeros((65, pk['N']), np.float32)
        c = 0
        for r in hs_rels:
            w[0:64, c:c + 64] = Ws[r] * _RELS[r]['scale']
            c += 64
        w[0:64, c:c + 64] = lin[t][0]
        w[64, c:c + 64] = bias[t]
        c += 64
        nv = len(vec_rels)
        half = nv // 2
        for i, r in enumerate(vec_rels):
            w[0:64, c + i] = wsv[r] if i < half else wdv[r]
        packs[t] = w.astype(BF16)
    return packs


def kernel(hex_x, vertex_x, road_x, params):
    nc = _get_nc()
    xs = {'h': _np(hex_x), 'v': _np(vertex_x), 'r': _np(road_x)}
    packs = _build_packs(params)

    in_maps = []
    for c in range(N_CORES):
        m = {}
        for t in ('h', 'v', 'r'):
            xc = xs[t][c * B_CORE:(c + 1) * B_CORE]        # [256, Ns, 64]
            a = np.empty((65, _NN[t] * B_CORE), np.float32)
            a[0:64] = xc.transpose(2, 1, 0).reshape(64, -1)
            a[64] = 1.0
            m[f'xt_{t}'] = a.astype(BF16)
            m[f'wp_{t}'] = packs[t]
        in_maps.append(m)

    res = run_bass_kernel_spmd(nc, in_maps, list(range(N_CORES))).results
    outs = {}
    for t, Ns in (('h', HEX_N), ('v', VER_N), ('r', ROAD_N)):
        outs[t] = np.concatenate(
            [np.asarray(res[c][f'out_{t}']).reshape(B_CORE, Ns, D)
             for c in range(N_CORES)], axis=0)
    return outs['h'], outs['v'], outs['r']
